# revision 25
# baseline (speedup 1.0000x reference)
"""Trainium2 Bass kernel for a single causal attention head.

Problem: x:(8,2048,1024) f32, per-head projections wq/wk/wv:(64,1024),
biases (64,). Output: softmax(causal(q k^T / sqrt(64))) @ v : (8,2048,64).

Strategy:
  - Data-parallel: batch b -> core b (8 cores, 1 batch each).
  - Host prep packs inputs into partition-major, fully contiguous
    per-partition byte streams, TWO dram tensors (one per HWDGE queue):
      [w1-half f16 | biases-or-wv | x fp8 (all chunks) | x f16 (all chunks)]
    The fp8 copy of x feeds only the Q/K projections (rel-err ~1.3e-2,
    under the 2e-2 gate); the f16 copy feeds the V projection. Streaming
    the small fp8 copy first lets the QK projections and all score/EXP
    work start ~3us earlier than waiting for f16 x; the f16 copy lands
    while scores stream.
  - Device (per core):
      * qk1 = [wq*s|wk]^T.T @ x8: rows 0-63 = Q^T, rows 64-127 = K^T (PSUM
        accumulate over 8 d-tiles, N=512 chunks).
      * qk2 = half-swapped copy of qk1 -> both Q^T and K^T live on both
        partition halves; scores for two k-tiles share the PE array via
        row packing.
      * vT (64,T) f16 from the f16 x, transposed back to (T,64) tiles via
        PE transpose, augmented with a ones column (softmax denominator
        rides along the PV matmul).
      * S^T = K^T.T @ Q^T per k-tile; P^T = exp(S^T) on ACT; causal mask
        via gpsimd affine_select restricted to the 128-col diagonal band.
      * Diagonal pairs run FIRST per chunk with column-trimmed scores/
        exp/mask/PV (fully-masked columns never computed); non-diagonal
        pairs follow full-range.
      * O^T_aug[65, T] accumulated in PSUM over k-tiles; row 64 = sum_j P^T.
  - Host post: out[b] = (O^T[0:64] / O^T[64:65]).T  (softmax normalization).
"""

import numpy as np

B, T, D, HD = 8, 2048, 1024, 64
P = 128          # SBUF partitions
CH = 512         # q-chunk (matmul moving dim)
NCH = T // CH    # 4
DT = D // P      # 8 d-tiles
NKT = T // P     # 16 k-tiles
NWARM = 8        # PE clock-ramp warmup matmuls
DH = DT // 2     # d-tiles per queue (d 0-3 on sync, 4-7 on scalar)

# per-partition byte offsets inside the two combined dram tensors.
# queue A (sync):   [w1A f16 | bb f32 | x8 c0..c3 (d0-3) | xf c0..c3 (d0-3)]
# queue B (scalar): [w1B f16 | wv f16 | x8 c0..c3 (d4-7) | xf c0..c3 (d4-7)]
W1BYT = DH * P * 2            # 1024: one w1 d-half
BBYT = 8                      # biases [P,2] f32
WVBYT = DT * HD * 2           # 1024: wv, all d-tiles
X8C = DH * CH                 # 2048: fp8 half-chunk
XFC = DH * CH * 2             # 4096: f16 half-chunk
A_X8 = W1BYT + BBYT           # 1032
B_X8 = W1BYT + WVBYT          # 2048
A_XF = A_X8 + NCH * X8C       # 9224
B_XF = B_X8 + NCH * X8C       # 10240
A_TOT = A_XF + NCH * XFC      # 25608
B_TOT = B_XF + NCH * XFC      # 26624

LAST_RESULTS = None      # BassKernelResults of the most recent run (for test.py)


def _build_module(legalize=True):
    import concourse.bass as bass
    import concourse.mybir as mybir
    from concourse.tile import TileContext

    from concourse.masks import make_identity
    F32 = mybir.dt.float32
    F16 = mybir.dt.float16
    F8 = mybir.dt.float8e4

    nc = bass.Bass("TRN2", target_bir_lowering=True)

    wxa = nc.dram_tensor("wxa", (P, A_TOT), F8, kind="ExternalInput")
    wxb = nc.dram_tensor("wxb", (P, B_TOT), F8, kind="ExternalInput")
    outT = nc.dram_tensor("outT", (HD + 1, T), F16, kind="ExternalOutput")

    with TileContext(nc) as tc:
        with (
            tc.tile_pool(name="const", bufs=1) as const,
            tc.tile_pool(name="acts", bufs=1) as acts,
            tc.tile_pool(name="proj_ps", bufs=2, space="PSUM") as proj_ps,
            tc.tile_pool(name="tr_ps", bufs=1, space="PSUM") as tr_ps,
            tc.tile_pool(name="s_ps", bufs=2, space="PSUM") as s_ps,
            tc.tile_pool(name="o_ps", bufs=1, space="PSUM") as o_ps,
            tc.tile_pool(name="pwork", bufs=16) as pwork,
            tc.tile_pool(name="owork", bufs=3) as owork,
        ):
            # ---- PE warm-up first: throwaway matmuls keep the PE busy
            # through its clock-ramp window so real matmuls run at full
            # speed. Gated only on the wscr memset, not on any DMA.
            wscr = const.tile([P, CH], F16, name="wscr")
            nc.vector.memset(wscr[:], 0.0)
            for wu in range(NWARM):
                pswu = proj_ps.tile([P, CH], F32, name="warm", tag="proj")
                nc.tensor.matmul(pswu[:], wscr[:, 0:P], wscr[:],
                                 start=True, stop=True)

            # ---- input DMAs, priority-ordered: [weights + fp8 chunk0]
            # first, then the remaining fp8 x chunk by chunk (feeds
            # QK+scores: the EXP chain starts early and never starves),
            # then the f16 x chunk by chunk (feeds the per-chunk V
            # projections, each needed progressively later). The d0-3
            # halves + all f16-a ride the sync HWDGE ring; the d4-7 fp8
            # halves ride scalar (done before the first EXP needs that
            # queue); the f16-b halves ride the gpsimd SWDGE ring, keeping
            # the scalar queue free for the EXP stream. ----
            wx_a = const.tile([P, A_TOT], F8, name="wx_a")
            wx_b = const.tile([P, B_TOT], F8, name="wx_b")
            def ld_x8a(ci):
                a0 = A_X8 + ci * X8C
                nc.sync.dma_start(out=wx_a[:, a0:a0 + X8C],
                                  in_=wxa[:, a0:a0 + X8C])

            def ld_xf(ci):
                a0 = A_XF + ci * XFC
                b0 = B_XF + ci * XFC
                nc.sync.dma_start(out=wx_a[:, a0:a0 + XFC],
                                  in_=wxa[:, a0:a0 + XFC])
                nc.sync.dma_start(out=wx_b[:, b0:b0 + XFC],
                                  in_=wxb[:, b0:b0 + XFC])

            nc.sync.dma_start(out=wx_a[:, 0:A_X8 + X8C],
                              in_=wxa[:, 0:A_X8 + X8C])
            nc.scalar.dma_start(out=wx_b[:, 0:B_X8 + X8C],
                                in_=wxb[:, 0:B_X8 + X8C])
            for ci in range(1, NCH):
                b0 = B_X8 + ci * X8C
                nc.scalar.dma_start(out=wx_b[:, b0:b0 + X8C],
                                    in_=wxb[:, b0:b0 + X8C])
            # tiny throwaway EXP: makes walrus emit the ACT table load for
            # the exp set NOW (scalar queue is otherwise idle until the
            # first scores pair), instead of serializing the ~2.7us load
            # in front of the first real EXP.
            scr1 = const.tile([1, 1], F16, name="scr1")
            nc.scalar.activation(scr1[:], scr1[:],
                                 mybir.ActivationFunctionType.Exp)
            # ALL f16 transfers ride the sync ring BEHIND the fp8 stream:
            # ring-FIFO guarantees they cannot steal bandwidth from the
            # critical fp8/QK chain (a second ring would round-robin with
            # it at packet granularity and starve it). The f16 chunk 0
            # slots in BEFORE the fp8 chunk 3 (chunk-3 scores have ~4us of
            # slack on the EXP chain, while the V/PV tail wants every f16
            # chunk as early as possible).
            ld_x8a(1); ld_x8a(2)
            ld_xf(0)
            ld_x8a(3)
            ld_xf(1); ld_xf(2); ld_xf(3)

            b_sb = wx_a[:, W1BYT:W1BYT + BBYT].bitcast(F32)  # [P, 2] f32

            def w1s(d):
                t = wx_a if d < DH else wx_b
                dd = d % DH
                return t[:, dd * P * 2:(dd + 1) * P * 2].bitcast(F16)

            def wvs(d):
                return wx_b[:, W1BYT + d * HD * 2:
                            W1BYT + (d + 1) * HD * 2].bitcast(F16)

            def x8s(ci, d):
                t, base = (wx_a, A_X8) if d < DH else (wx_b, B_X8)
                dd = d % DH
                off = base + ci * X8C + dd * CH
                return t[:, off:off + CH]

            def xfs(ci, d):
                t, base = (wx_a, A_XF) if d < DH else (wx_b, B_XF)
                dd = d % DH
                off = base + ci * XFC + dd * CH * 2
                return t[:, off:off + CH * 2].bitcast(F16)

            ident = const.tile([P, P], F16, name="ident")
            make_identity(nc, ident)

            # ---- activations ----
            # qk1: rows 0-63 = Q^T, rows 64-127 = K^T; qk2: swapped halves.
            qk1 = acts.tile([P, T], F16, name="qk1")
            qk2 = acts.tile([P, T], F16, name="qk2")
            vT = acts.tile([HD, T], F16, name="vT")
            v_aug = acts.tile([P, NKT, HD + 1], F16, name="v_aug")
            nc.vector.memset(v_aug[:, :, HD], 1.0)

            def qk_chunk(ci):
                cs = slice(ci * CH, (ci + 1) * CH)
                ps = proj_ps.tile([P, CH], F32, name="proj", tag="proj")
                for d in range(DT):
                    nc.tensor.matmul(ps[:], w1s(d), x8s(ci, d),
                                     start=(d == 0), stop=(d == DT - 1))
                nc.vector.tensor_scalar_add(qk1[:, cs], ps[:], b_sb[:, 0:1])
                # half-swapped copy: qk2 = [K^T; Q^T]. 64-partition DVE ops
                # read any aligned src half and write either dest half.
                nc.vector.tensor_copy(qk2[0:HD, cs], qk1[HD:P, cs])
                nc.vector.tensor_copy(qk2[HD:P, cs], qk1[0:HD, cs])

            def v_mm(ca, inter=()):
                # V projection for one chunk (solo: the per-chunk f16 x
                # transfers land ~4us apart, so pairing two chunks would
                # couple each V block to the LATER chunk's data). `inter`
                # maps d-index -> thunk emitted after that d-step (scores
                # pairs interleaved so the EXP stream never starves while
                # the PE chews through the projection block).
                psv = proj_ps.tile([P, CH], F32, name="projv", tag="proj")
                for d in range(DT):
                    nc.tensor.matmul(psv[0:HD, :], wvs(d), xfs(ca, d),
                                     start=(d == 0), stop=(d == DT - 1))
                    if d in inter:
                        inter[d]()
                nc.vector.tensor_scalar_add(
                    vT[:, ca * CH:(ca + 1) * CH], psv[0:HD, :], b_sb[0:HD, 1:2])

            def v_tr(ca):
                for tt in range(4 * ca, 4 * ca + 4):
                    tp = tr_ps.tile([P, HD], F16, name="vtr", tag="vtr")
                    nc.tensor.transpose(tp[:], vT[:, tt * P:(tt + 1) * P],
                                        ident[:HD, :HD])
                    nc.vector.tensor_copy(v_aug[:, tt, 0:HD], tp[:])

            def chunk_pairs(ci):
                # diagonal pairs first (col-trimmed, masked), then full pairs
                return ([(4 * ci, 4 * ci + 1), (4 * ci + 2, 4 * ci + 3)]
                        + [(2 * j, 2 * j + 1) for j in range(2 * ci)])

            def scores_pair(ci, ka, kb, diag):
                c0 = ci * CH
                da = max(ka * P - c0, 0)  # first unmasked column
                db = max(kb * P - c0, 0)
                s2 = s_ps.tile([P, 2 * CH], F32, name="sT", tag="sT")
                # rows 0-63 of the array: K^T from qk2, Q^T from qk1
                nc.tensor.matmul(s2[:, da:CH],
                                 qk2[0:HD, ka * P:(ka + 1) * P],
                                 qk1[0:HD, c0 + da:c0 + CH],
                                 start=True, stop=True)
                # rows 64-127: K^T from qk1, Q^T from qk2 (concurrent)
                nc.tensor.matmul(s2[:, CH + db:2 * CH],
                                 qk1[HD:P, kb * P:(kb + 1) * P],
                                 qk2[HD:P, c0 + db:c0 + CH],
                                 start=True, stop=True)
                pt = pwork.tile([P, 2 * CH], F16, name="pT", tag="pT")
                if diag:
                    if db <= P:
                        # single EXP over [da:2CH]: the gap cols [CH:CH+db]
                        # are stale-PSUM garbage exp'd harmlessly (never
                        # consumed); one instruction saves ~290ns of ACT
                        # fixed cost, more than the db extra columns cost.
                        nc.scalar.activation(pt[:, da:2 * CH],
                                             s2[:, da:2 * CH],
                                             mybir.ActivationFunctionType.Exp)
                    else:
                        nc.scalar.activation(pt[:, da:CH], s2[:, da:CH],
                                             mybir.ActivationFunctionType.Exp)
                        nc.scalar.activation(pt[:, CH + db:2 * CH],
                                             s2[:, CH + db:2 * CH],
                                             mybir.ActivationFunctionType.Exp)
                    # causal mask on the 128-col diagonal band only:
                    # keep where (query - delta) >= key  <=>  c' >= p
                    for off in (da, CH + db):
                        nc.gpsimd.affine_select(
                            out=pt[:, off:off + P],
                            in_=pt[:, off:off + P],
                            compare_op=mybir.AluOpType.is_ge, fill=0.0,
                            base=0, pattern=[[1, P]],
                            channel_multiplier=-1,
                        )
                else:
                    nc.scalar.activation(pt[:], s2[:],
                                         mybir.ActivationFunctionType.Exp)
                return pt

            def pv_pair(ci, ops, ka, kb, pt, first, last):
                c0 = ci * CH
                da = max(ka * P - c0, 0)
                db = max(kb * P - c0, 0)
                nc.tensor.matmul(ops[:, da:CH], v_aug[:, ka, :],
                                 pt[:, da:CH],
                                 start=first, stop=False)
                nc.tensor.matmul(ops[:, db:CH], v_aug[:, kb, :],
                                 pt[:, CH + db:2 * CH],
                                 start=False, stop=last)

            def store_chunk(ci, ops):
                # f16 output (error budget << 2e-2 gate) halves the
                # store transfers; the host divides in f32.
                osb = owork.tile([HD + 1, CH], F16, name="osb", tag="osb")
                nc.vector.tensor_copy(osb[:], ops[:])
                nc.sync.dma_start(
                    out=outT[:, ci * CH:(ci + 1) * CH], in_=osb[:])

            # ---- global software pipeline ----
            # The fp8 QK stream lands chunk-by-chunk ~1.4us apart, so all
            # four QK projections run front-to-back; scores stream behind
            # them at EXP-drain rate; the f16 V stream lands mid-flight and
            # the V-projection / transpose blocks act as PE filler between
            # score pairs; PV lags behind its chunk's scores.
            pts = {}
            opses = {}

            def sc(ci, j):
                ka, kb = chunk_pairs(ci)[j]
                pts[(ci, j)] = scores_pair(ci, ka, kb, diag=j < 2)

            def pv(ci, j):
                pairs = chunk_pairs(ci)
                ka, kb = pairs[j]
                pv_pair(ci, opses[ci], ka, kb, pts.pop((ci, j)),
                        first=j == 0, last=j == len(pairs) - 1)

            M = CH // 2

            def pv3(j, phase, start=False, stop=False, pop=False):
                # chunk 3's accumulator is split into column halves living
                # in two PSUM banks, each filled by its own phase pass, so
                # the a-half's copy+store and the b-half's matmuls overlap;
                # the full pairs (j>=2, k-tiles 0-11) run as soon as their
                # EXPs land, while only the two diagonal pairs (k-tiles
                # 12-15) wait for the late V projection of chunk 3.
                ka, kb = chunk_pairs(3)[j]
                pt = pts.pop((3, j)) if pop else pts[(3, j)]
                c0 = 3 * CH
                da = max(ka * P - c0, 0)
                db = max(kb * P - c0, 0)
                oa, ob = opses[3]
                if phase == "a":
                    mms = []
                    if da < M:
                        mms.append((oa[:, da:M], ka, pt[:, da:M]))
                    if db < M:
                        mms.append((oa[:, db:M], kb, pt[:, CH + db:CH + M]))
                else:
                    ba = max(da, M)
                    bb = max(db, M)
                    mms = [(ob[:, ba - M:M], ka, pt[:, ba:CH]),
                           (ob[:, bb - M:M], kb, pt[:, CH + bb:2 * CH])]
                for i, (o, kt, p) in enumerate(mms):
                    nc.tensor.matmul(o, v_aug[:, kt, :], p,
                                     start=start and i == 0,
                                     stop=stop and i == len(mms) - 1)

            qk_chunk(0)
            sc(0, 0); sc(0, 1)
            qk_chunk(1)
            sc(1, 0); sc(1, 1)
            qk_chunk(2)
            sc(1, 2); sc(1, 3)
            qk_chunk(3)
            sc(2, 0); sc(2, 1)
            v_mm(0, inter={3: lambda: sc(2, 2), 6: lambda: sc(2, 3)})
            v_tr(0)
            sc(2, 4)
            opses[0] = o_ps.tile([HD + 1, CH], F32, name="oacc", tag="oacc")
            pv(0, 0); pv(0, 1)
            sc(2, 5)
            store_chunk(0, opses[0])
            v_mm(1, inter={3: lambda: sc(3, 0), 6: lambda: sc(3, 1)})
            v_tr(1)
            sc(3, 2)
            opses[1] = o_ps.tile([HD + 1, CH], F32, name="oacc", tag="oacc")
            pv(1, 0); pv(1, 1)
            sc(3, 3)
            pv(1, 2); pv(1, 3)
            store_chunk(1, opses[1])
            v_mm(2, inter={3: lambda: sc(3, 4), 6: lambda: sc(3, 5)})
            v_tr(2)
            sc(3, 6)
            opses[2] = o_ps.tile([HD + 1, CH], F32, name="oacc", tag="oacc")
            pv(2, 0); pv(2, 1)
            # chunk 3's a-half accumulator borrows a projection-pool bank
            # (the pool's rotation frees one after v_mm(1)); its b-half
            # borrows the transpose pool's bank after v_tr(3). That lets
            # the full pairs' PV matmuls run during the EXP-chain window
            # instead of serializing behind the chunk-3 V projection.
            o3a = proj_ps.tile([HD + 1, M], F32, name="oacc3a", tag="proj")
            opses[3] = [o3a, None]
            pv(2, 2); pv(2, 3)
            sc(3, 7)
            pv3(2, "a", start=True); pv3(3, "a")
            pv(2, 4)
            pv3(4, "a"); pv3(5, "a")
            pv(2, 5)
            store_chunk(2, opses[2])
            v_mm(3)
            v_tr(3)
            opses[3][1] = tr_ps.tile([HD + 1, M], F32, name="oacc3b",
                                     tag="vtr")
            pv3(2, "b", start=True); pv3(3, "b"); pv3(4, "b"); pv3(5, "b")
            pv3(6, "a"); pv3(6, "b")
            pv3(7, "a"); pv3(7, "b")
            # diagonal pairs last: finish the a-half first so its
            # copy+store (slow scalar-queue trigger) launches while the
            # b-half's final PV matmuls still run
            pv3(0, "a", stop=True)
            osb_a = owork.tile([HD + 1, M], F16, name="osb3a", tag="osb")
            nc.vector.tensor_copy(osb_a[:], opses[3][0][:])
            nc.scalar.dma_start(out=outT[:, 3 * CH:3 * CH + M],
                                in_=osb_a[:])
            pv3(0, "b", pop=True)
            pv3(1, "b", stop=True, pop=True)
            osb_b = owork.tile([HD + 1, M], F16, name="osb3b", tag="osb")
            nc.vector.tensor_copy(osb_b[:], opses[3][1][:])
            nc.sync.dma_start(out=outT[:, 3 * CH + M:4 * CH],
                              in_=osb_b[:])

    if legalize:
        _legalize_waits(nc, mybir)
    return nc


def _legalize_waits(nc, mybir):
    """Split multi-wait instructions for the XLA-route walrus codegen.

    The TPB EVENTS struct holds one semaphore wait per instruction and this
    pipeline's codegen refuses >1. Hoist extra waits onto standalone
    EventSemaphore instructions on the same engine queue right before the
    instruction - semantically identical, the queue stalls there.
    """
    n = 0
    for f in nc.m.functions:
        for b in f.blocks:
            out = []
            changed = False
            for inst in b.instructions:
                si = inst.sync_info
                waits = list(si.on_wait) if si is not None and si.on_wait else []
                if len(waits) > 1:
                    changed = True
                    for w in waits[:-1]:
                        n += 1
                        out.append(mybir.InstEventSemaphore(
                            name=f"waitfix{n}_{inst.name}",
                            engine=inst.engine,
                            sync_info=mybir.SyncInfo(on_wait=[w], on_update=[]),
                        ))
                    inst.sync_info = mybir.SyncInfo(
                        on_wait=waits[-1:],
                        on_update=list(si.on_update or []),
                    )
                out.append(inst)
            if changed:
                b.instructions = out
    return n


def kernel(x, wq, bq, wk, bk, wv, bv):
    global LAST_RESULTS
    import os
    os.environ.setdefault("JAX_PLATFORMS", "")
    import ml_dtypes
    from concourse.bass_utils import run_bass_kernel_spmd

    FP8 = ml_dtypes.float8_e4m3
    x = np.asarray(x, dtype=np.float32)
    s = np.float32(1.0 / np.sqrt(HD))
    # per partition p (= row of the D-contraction tile), d-major columns
    w1 = np.concatenate([np.asarray(wq, np.float32) * s,
                         np.asarray(wk, np.float32)], 0).T  # (D, 128)
    w1d = np.ascontiguousarray(
        w1.reshape(DT, P, P).transpose(1, 0, 2)
        .reshape(P, DT * P)).astype(np.float16)
    wv_t = np.asarray(wv, np.float32).T                      # (D, 64)
    wvd = np.ascontiguousarray(
        wv_t.reshape(DT, P, HD).transpose(1, 0, 2)
        .reshape(P, DT * HD)).astype(np.float16)
    b1 = np.concatenate([np.asarray(bq, np.float32) * s,
                         np.asarray(bk, np.float32)])
    bv_f = np.asarray(bv, np.float32)
    bb = np.ascontiguousarray(
        np.stack([b1, np.concatenate([bv_f, bv_f])], axis=1))  # (P, 2)
    # xp[b]: partition-major, chunk-major, d-major: row p holds, for each
    # chunk ci and d-tile d, the 512 values x[b, ci*CH:(ci+1)*CH, d*P+p].
    xp32 = np.ascontiguousarray(
        x.reshape(B, NCH, CH, DT, P).transpose(0, 4, 1, 3, 2)
        .reshape(B, P, NCH, DT, CH))          # (B, P, ci, d, CH) f32
    xp8 = xp32.astype(FP8)
    xp16 = xp32.astype(np.float16)

    def v8(a):
        return np.ascontiguousarray(a).view(FP8).reshape(P, -1)

    wxa_b, wxb_b = [], []
    for b in range(B):
        x8a = v8(xp8[b, :, :, :DH])           # chunks-major, d 0-3
        x8b = v8(xp8[b, :, :, DH:])
        xfa = v8(xp16[b, :, :, :DH])
        xfb = v8(xp16[b, :, :, DH:])
        wxa_b.append(np.ascontiguousarray(np.concatenate(
            [v8(w1d[:, :DH * P]), v8(bb), x8a, xfa], axis=1)))
        wxb_b.append(np.ascontiguousarray(np.concatenate(
            [v8(w1d[:, DH * P:]), v8(wvd), x8b, xfb], axis=1)))
    assert wxa_b[0].shape == (P, A_TOT), wxa_b[0].shape
    assert wxb_b[0].shape == (P, B_TOT), wxb_b[0].shape

    nc = _build_module()
    in_maps = [
        {"wxa": wxa_b[b], "wxb": wxb_b[b]}
        for b in range(B)
    ]
    res = None
    for attempt in range(3):
        try:
            res = run_bass_kernel_spmd(nc, in_maps, core_ids=list(range(B)))
            break
        except Exception:
            # transient device wedges (NRT_EXEC_UNIT_UNRECOVERABLE) happen;
            # rebuild the module and retry on a clean execution
            if attempt == 2:
                raise
            nc = _build_module()
    LAST_RESULTS = res

    out = np.empty((B, T, HD), dtype=np.float32)
    for b in range(B):
        # (65, T) f16: rows 0..63 = O^T, row 64 = denom; divide in f32
        oT = np.asarray(res.results[b]["outT"], dtype=np.float32)
        out[b] = (oT[:HD] / oT[HD:HD + 1]).T
    return out


# revision 26
# speedup vs baseline: 1.2210x; 1.2210x over previous
"""Trainium2 Bass kernel for a single causal attention head.

Problem: x:(8,2048,1024) f32, per-head projections wq/wk/wv:(64,1024),
biases (64,). Output: softmax(causal(q k^T / sqrt(64))) @ v : (8,2048,64).

Strategy:
  - Data-parallel: batch b -> core b (8 cores, 1 batch each).
  - Host prep packs inputs into partition-major, fully contiguous
    per-partition byte streams, TWO dram tensors (one per HWDGE queue):
      [w1-half f16 | biases-or-wv | x fp8 (all chunks) | x f16 (all chunks)]
    The fp8 copy of x feeds only the Q/K projections (rel-err ~1.3e-2,
    under the 2e-2 gate); the f16 copy feeds the V projection. Streaming
    the small fp8 copy first lets the QK projections and all score/EXP
    work start ~3us earlier than waiting for f16 x; the f16 copy lands
    while scores stream.
  - Device (per core):
      * qk1 = [wq*s|wk]^T.T @ x8: rows 0-63 = Q^T, rows 64-127 = K^T (PSUM
        accumulate over 8 d-tiles, N=512 chunks).
      * qk2 = half-swapped copy of qk1 -> both Q^T and K^T live on both
        partition halves; scores for two k-tiles share the PE array via
        row packing.
      * vT (64,T) f16 from the f16 x, transposed back to (T,64) tiles via
        PE transpose, augmented with a ones column (softmax denominator
        rides along the PV matmul).
      * S^T = K^T.T @ Q^T per k-tile; P^T = exp(S^T) on ACT; causal mask
        via gpsimd affine_select restricted to the 128-col diagonal band.
      * Diagonal pairs run FIRST per chunk with column-trimmed scores/
        exp/mask/PV (fully-masked columns never computed); non-diagonal
        pairs follow full-range.
      * O^T_aug[65, T] accumulated in PSUM over k-tiles; row 64 = sum_j P^T.
  - Host post: out[b] = (O^T[0:64] / O^T[64:65]).T  (softmax normalization).
"""

import numpy as np

B, T, D, HD = 8, 2048, 1024, 64
P = 128          # SBUF partitions
CH = 512         # q-chunk (matmul moving dim)
NCH = T // CH    # 4
DT = D // P      # 8 d-tiles
NKT = T // P     # 16 k-tiles
NWARM = 8        # PE clock-ramp warmup matmuls
DH = DT // 2     # d-tiles per queue (d 0-3 on sync, 4-7 on scalar)

# per-partition byte offsets inside the two combined dram tensors.
# queue A (sync):   [w1A f16 | bb f32 | x8 c0..c3 (d0-3) | xf c0..c3 (d0-3)]
# queue B (scalar): [w1B f16 | wv f16 | x8 c0..c3 (d4-7) | xf c0..c3 (d4-7)]
W1BYT = DH * P * 2            # 1024: one w1 d-half
BBYT = 8                      # biases [P,2] f32
WVBYT = DT * HD * 2           # 1024: wv, all d-tiles
X8C = DH * CH                 # 2048: fp8 half-chunk
XFC = DH * CH * 2             # 4096: f16 half-chunk
A_X8 = W1BYT + BBYT           # 1032
B_X8 = W1BYT + WVBYT          # 2048
A_XF = A_X8 + NCH * X8C       # 9224
B_XF = B_X8 + NCH * X8C       # 10240
A_TOT = A_XF + NCH * XFC      # 25608
B_TOT = B_XF + NCH * XFC      # 26624

LAST_RESULTS = None      # BassKernelResults of the most recent run (for test.py)


def _build_module(legalize=True):
    import concourse.bass as bass
    import concourse.mybir as mybir
    from concourse.tile import TileContext

    from concourse.masks import make_identity
    F32 = mybir.dt.float32
    F16 = mybir.dt.float16
    F8 = mybir.dt.float8e4

    nc = bass.Bass("TRN2", target_bir_lowering=True)

    wxa = nc.dram_tensor("wxa", (P, A_TOT), F8, kind="ExternalInput")
    wxb = nc.dram_tensor("wxb", (P, B_TOT), F8, kind="ExternalInput")
    outT = nc.dram_tensor("outT", (HD + 1, T), F16, kind="ExternalOutput")

    with TileContext(nc) as tc:
        with (
            tc.tile_pool(name="const", bufs=1) as const,
            tc.tile_pool(name="acts", bufs=1) as acts,
            tc.tile_pool(name="proj_ps", bufs=2, space="PSUM") as proj_ps,
            tc.tile_pool(name="tr_ps", bufs=1, space="PSUM") as tr_ps,
            tc.tile_pool(name="s_ps", bufs=2, space="PSUM") as s_ps,
            tc.tile_pool(name="o_ps", bufs=1, space="PSUM") as o_ps,
            tc.tile_pool(name="pwork", bufs=16) as pwork,
            tc.tile_pool(name="owork", bufs=3) as owork,
        ):
            # ---- PE warm-up first: throwaway matmuls keep the PE busy
            # through its clock-ramp window so real matmuls run at full
            # speed. Gated only on the wscr memset, not on any DMA.
            wscr = const.tile([P, CH], F16, name="wscr")
            nc.vector.memset(wscr[:], 0.0)
            for wu in range(NWARM):
                pswu = proj_ps.tile([P, CH], F32, name="warm", tag="proj")
                nc.tensor.matmul(pswu[:], wscr[:, 0:P], wscr[:],
                                 start=True, stop=True)

            # ---- input DMAs, priority-ordered: [weights + fp8 chunk0]
            # first, then the remaining fp8 x chunk by chunk (feeds
            # QK+scores: the EXP chain starts early and never starves),
            # then the f16 x chunk by chunk (feeds the per-chunk V
            # projections, each needed progressively later). The d0-3
            # halves + all f16-a ride the sync HWDGE ring; the d4-7 fp8
            # halves ride scalar (done before the first EXP needs that
            # queue); the f16-b halves ride the gpsimd SWDGE ring, keeping
            # the scalar queue free for the EXP stream. ----
            wx_a = const.tile([P, A_TOT], F8, name="wx_a")
            wx_b = const.tile([P, B_TOT], F8, name="wx_b")
            def ld_x8a(ci):
                a0 = A_X8 + ci * X8C
                nc.sync.dma_start(out=wx_a[:, a0:a0 + X8C],
                                  in_=wxa[:, a0:a0 + X8C])

            def ld_xf(ci):
                a0 = A_XF + ci * XFC
                b0 = B_XF + ci * XFC
                nc.sync.dma_start(out=wx_a[:, a0:a0 + XFC],
                                  in_=wxa[:, a0:a0 + XFC])
                nc.sync.dma_start(out=wx_b[:, b0:b0 + XFC],
                                  in_=wxb[:, b0:b0 + XFC])

            nc.sync.dma_start(out=wx_a[:, 0:A_X8 + X8C],
                              in_=wxa[:, 0:A_X8 + X8C])
            nc.scalar.dma_start(out=wx_b[:, 0:B_X8 + X8C],
                                in_=wxb[:, 0:B_X8 + X8C])
            for ci in range(1, NCH):
                b0 = B_X8 + ci * X8C
                nc.scalar.dma_start(out=wx_b[:, b0:b0 + X8C],
                                    in_=wxb[:, b0:b0 + X8C])
            # tiny throwaway EXP: makes walrus emit the ACT table load for
            # the exp set NOW (scalar queue is otherwise idle until the
            # first scores pair), instead of serializing the ~2.7us load
            # in front of the first real EXP.
            scr1 = const.tile([1, 1], F16, name="scr1")
            nc.scalar.activation(scr1[:], scr1[:],
                                 mybir.ActivationFunctionType.Exp)
            # ALL f16 transfers ride the sync ring BEHIND the fp8 stream:
            # ring-FIFO guarantees they cannot steal bandwidth from the
            # critical fp8/QK chain (a second ring would round-robin with
            # it at packet granularity and starve it). The f16 chunk 0
            # slots in BEFORE the fp8 chunk 3 (chunk-3 scores have ~4us of
            # slack on the EXP chain, while the V/PV tail wants every f16
            # chunk as early as possible).
            ld_x8a(1); ld_x8a(2)
            ld_xf(0)
            ld_x8a(3)
            ld_xf(1); ld_xf(2); ld_xf(3)

            b_sb = wx_a[:, W1BYT:W1BYT + BBYT].bitcast(F32)  # [P, 2] f32

            def w1s(d):
                t = wx_a if d < DH else wx_b
                dd = d % DH
                return t[:, dd * P * 2:(dd + 1) * P * 2].bitcast(F16)

            def wvs(d):
                return wx_b[:, W1BYT + d * HD * 2:
                            W1BYT + (d + 1) * HD * 2].bitcast(F16)

            def x8s(ci, d):
                t, base = (wx_a, A_X8) if d < DH else (wx_b, B_X8)
                dd = d % DH
                off = base + ci * X8C + dd * CH
                return t[:, off:off + CH]

            def xfs(ci, d):
                t, base = (wx_a, A_XF) if d < DH else (wx_b, B_XF)
                dd = d % DH
                off = base + ci * XFC + dd * CH * 2
                return t[:, off:off + CH * 2].bitcast(F16)

            ident = const.tile([P, P], F16, name="ident")
            make_identity(nc, ident)

            # ---- activations ----
            # qk1: rows 0-63 = Q^T, rows 64-127 = K^T; qk2: swapped halves.
            qk1 = acts.tile([P, T], F16, name="qk1")
            qk2 = acts.tile([P, T], F16, name="qk2")
            vT = acts.tile([HD, T], F16, name="vT")
            v_aug = acts.tile([P, NKT, HD + 1], F16, name="v_aug")
            nc.vector.memset(v_aug[:, :, HD], 1.0)

            def qk_chunk(ci):
                cs = slice(ci * CH, (ci + 1) * CH)
                ps = proj_ps.tile([P, CH], F32, name="proj", tag="proj")
                for d in range(DT):
                    nc.tensor.matmul(ps[:], w1s(d), x8s(ci, d),
                                     start=(d == 0), stop=(d == DT - 1))
                nc.vector.tensor_scalar_add(qk1[:, cs], ps[:], b_sb[:, 0:1])
                # half-swapped copy: qk2 = [K^T; Q^T]. 64-partition DVE ops
                # read any aligned src half and write either dest half.
                nc.vector.tensor_copy(qk2[0:HD, cs], qk1[HD:P, cs])
                nc.vector.tensor_copy(qk2[HD:P, cs], qk1[0:HD, cs])

            def v_mm(ca, inter=()):
                # V projection for one chunk (solo: the per-chunk f16 x
                # transfers land ~4us apart, so pairing two chunks would
                # couple each V block to the LATER chunk's data). `inter`
                # maps d-index -> thunk emitted after that d-step (scores
                # pairs interleaved so the EXP stream never starves while
                # the PE chews through the projection block).
                psv = proj_ps.tile([P, CH], F32, name="projv", tag="proj")
                for d in range(DT):
                    nc.tensor.matmul(psv[0:HD, :], wvs(d), xfs(ca, d),
                                     start=(d == 0), stop=(d == DT - 1))
                    if d in inter:
                        inter[d]()
                nc.vector.tensor_scalar_add(
                    vT[:, ca * CH:(ca + 1) * CH], psv[0:HD, :], b_sb[0:HD, 1:2])

            def v_tr(ca):
                for tt in range(4 * ca, 4 * ca + 4):
                    tp = tr_ps.tile([P, HD], F16, name="vtr", tag="vtr")
                    nc.tensor.transpose(tp[:], vT[:, tt * P:(tt + 1) * P],
                                        ident[:HD, :HD])
                    nc.vector.tensor_copy(v_aug[:, tt, 0:HD], tp[:])

            def chunk_pairs(ci):
                # diagonal pairs first (col-trimmed, masked), then full pairs
                return ([(4 * ci, 4 * ci + 1), (4 * ci + 2, 4 * ci + 3)]
                        + [(2 * j, 2 * j + 1) for j in range(2 * ci)])

            def scores_pair(ci, ka, kb, diag):
                c0 = ci * CH
                da = max(ka * P - c0, 0)  # first unmasked column
                db = max(kb * P - c0, 0)
                s2 = s_ps.tile([P, 2 * CH], F32, name="sT", tag="sT")
                # rows 0-63 of the array: K^T from qk2, Q^T from qk1
                nc.tensor.matmul(s2[:, da:CH],
                                 qk2[0:HD, ka * P:(ka + 1) * P],
                                 qk1[0:HD, c0 + da:c0 + CH],
                                 start=True, stop=True)
                # rows 64-127: K^T from qk1, Q^T from qk2 (concurrent)
                nc.tensor.matmul(s2[:, CH + db:2 * CH],
                                 qk1[HD:P, kb * P:(kb + 1) * P],
                                 qk2[HD:P, c0 + db:c0 + CH],
                                 start=True, stop=True)
                pt = pwork.tile([P, 2 * CH], F16, name="pT", tag="pT")
                if diag:
                    if db <= P:
                        # single EXP over [da:2CH]: the gap cols [CH:CH+db]
                        # are stale-PSUM garbage exp'd harmlessly (never
                        # consumed); one instruction saves ~290ns of ACT
                        # fixed cost, more than the db extra columns cost.
                        nc.scalar.activation(pt[:, da:2 * CH],
                                             s2[:, da:2 * CH],
                                             mybir.ActivationFunctionType.Exp)
                    else:
                        nc.scalar.activation(pt[:, da:CH], s2[:, da:CH],
                                             mybir.ActivationFunctionType.Exp)
                        nc.scalar.activation(pt[:, CH + db:2 * CH],
                                             s2[:, CH + db:2 * CH],
                                             mybir.ActivationFunctionType.Exp)
                    # causal mask on the 128-col diagonal band only:
                    # keep where (query - delta) >= key  <=>  c' >= p
                    for off in (da, CH + db):
                        nc.gpsimd.affine_select(
                            out=pt[:, off:off + P],
                            in_=pt[:, off:off + P],
                            compare_op=mybir.AluOpType.is_ge, fill=0.0,
                            base=0, pattern=[[1, P]],
                            channel_multiplier=-1,
                        )
                else:
                    nc.scalar.activation(pt[:], s2[:],
                                         mybir.ActivationFunctionType.Exp)
                return pt

            def pv_pair(ci, ops, ka, kb, pt, first, last):
                c0 = ci * CH
                da = max(ka * P - c0, 0)
                db = max(kb * P - c0, 0)
                nc.tensor.matmul(ops[:, da:CH], v_aug[:, ka, :],
                                 pt[:, da:CH],
                                 start=first, stop=False)
                nc.tensor.matmul(ops[:, db:CH], v_aug[:, kb, :],
                                 pt[:, CH + db:2 * CH],
                                 start=False, stop=last)

            def store_chunk(ci, ops):
                # f16 output (error budget << 2e-2 gate) halves the
                # store transfers; the host divides in f32.
                osb = owork.tile([HD + 1, CH], F16, name="osb", tag="osb")
                nc.vector.tensor_copy(osb[:], ops[:])
                nc.sync.dma_start(
                    out=outT[:, ci * CH:(ci + 1) * CH], in_=osb[:])

            # ---- global software pipeline ----
            # The fp8 QK stream lands chunk-by-chunk ~1.4us apart, so all
            # four QK projections run front-to-back; scores stream behind
            # them at EXP-drain rate; the f16 V stream lands mid-flight and
            # the V-projection / transpose blocks act as PE filler between
            # score pairs; PV lags behind its chunk's scores.
            pts = {}
            opses = {}

            def sc(ci, j):
                ka, kb = chunk_pairs(ci)[j]
                pts[(ci, j)] = scores_pair(ci, ka, kb, diag=j < 2)

            def pv(ci, j):
                pairs = chunk_pairs(ci)
                ka, kb = pairs[j]
                pv_pair(ci, opses[ci], ka, kb, pts.pop((ci, j)),
                        first=j == 0, last=j == len(pairs) - 1)

            M = CH // 2

            def pv3(j, phase, start=False, stop=False, pop=False):
                # chunk 3's accumulator is split into column halves living
                # in two PSUM banks, each filled by its own phase pass, so
                # the a-half's copy+store and the b-half's matmuls overlap;
                # the full pairs (j>=2, k-tiles 0-11) run as soon as their
                # EXPs land, while only the two diagonal pairs (k-tiles
                # 12-15) wait for the late V projection of chunk 3.
                ka, kb = chunk_pairs(3)[j]
                pt = pts.pop((3, j)) if pop else pts[(3, j)]
                c0 = 3 * CH
                da = max(ka * P - c0, 0)
                db = max(kb * P - c0, 0)
                oa, ob = opses[3]
                if phase == "a":
                    mms = []
                    if da < M:
                        mms.append((oa[:, da:M], ka, pt[:, da:M]))
                    if db < M:
                        mms.append((oa[:, db:M], kb, pt[:, CH + db:CH + M]))
                else:
                    ba = max(da, M)
                    bb = max(db, M)
                    mms = [(ob[:, ba - M:M], ka, pt[:, ba:CH]),
                           (ob[:, bb - M:M], kb, pt[:, CH + bb:2 * CH])]
                for i, (o, kt, p) in enumerate(mms):
                    nc.tensor.matmul(o, v_aug[:, kt, :], p,
                                     start=start and i == 0,
                                     stop=stop and i == len(mms) - 1)

            qk_chunk(0)
            sc(0, 0); sc(0, 1)
            qk_chunk(1)
            sc(1, 0); sc(1, 1)
            qk_chunk(2)
            sc(1, 2); sc(1, 3)
            qk_chunk(3)
            sc(2, 0); sc(2, 1)
            v_mm(0, inter={3: lambda: sc(2, 2), 6: lambda: sc(2, 3)})
            v_tr(0)
            sc(2, 4)
            opses[0] = o_ps.tile([HD + 1, CH], F32, name="oacc", tag="oacc")
            pv(0, 0); pv(0, 1)
            sc(2, 5)
            store_chunk(0, opses[0])
            v_mm(1, inter={3: lambda: sc(3, 0), 6: lambda: sc(3, 1)})
            v_tr(1)
            sc(3, 2)
            opses[1] = o_ps.tile([HD + 1, CH], F32, name="oacc", tag="oacc")
            pv(1, 0); pv(1, 1)
            sc(3, 3)
            pv(1, 2); pv(1, 3)
            store_chunk(1, opses[1])
            v_mm(2, inter={3: lambda: sc(3, 4), 6: lambda: sc(3, 5)})
            v_tr(2)
            opses[2] = o_ps.tile([HD + 1, CH], F32, name="oacc", tag="oacc")
            pv(2, 0); pv(2, 1)
            # chunk 3's a-half accumulator borrows a projection-pool bank
            # (the pool's rotation frees one after v_mm(1)); its b-half
            # borrows the transpose pool's bank after v_tr(3). That lets
            # the full pairs' PV matmuls run during the EXP-chain window
            # instead of serializing behind the chunk-3 V projection.
            o3a = proj_ps.tile([HD + 1, M], F32, name="oacc3a", tag="proj")
            opses[3] = [o3a, None]
            sc(3, 6)
            pv(2, 2); pv(2, 3)
            pv3(2, "a", start=True); pv3(3, "a")
            # chunk-3 V projection emitted BEFORE anything gated on the
            # last EXPs, so the PE FIFO doesn't serialize it behind them
            v_mm(3)
            v_tr(3)
            opses[3][1] = tr_ps.tile([HD + 1, M], F32, name="oacc3b",
                                     tag="vtr")
            sc(3, 7)
            pv3(4, "a"); pv3(5, "a")
            pv(2, 4)
            pv3(2, "b", start=True); pv3(3, "b")
            pv(2, 5)
            store_chunk(2, opses[2])
            pv3(4, "b"); pv3(5, "b")
            pv3(6, "a"); pv3(6, "b")
            pv3(7, "a"); pv3(7, "b")
            # diagonal pairs last: finish the a-half first so its
            # copy+store (slow scalar-queue trigger) launches while the
            # b-half's final PV matmuls still run
            pv3(0, "a", stop=True)
            osb_a = owork.tile([HD + 1, M], F16, name="osb3a", tag="osb")
            nc.vector.tensor_copy(osb_a[:], opses[3][0][:])
            nc.scalar.dma_start(out=outT[:, 3 * CH:3 * CH + M],
                                in_=osb_a[:])
            pv3(0, "b", pop=True)
            pv3(1, "b", stop=True, pop=True)
            osb_b = owork.tile([HD + 1, M], F16, name="osb3b", tag="osb")
            nc.vector.tensor_copy(osb_b[:], opses[3][1][:])
            nc.sync.dma_start(out=outT[:, 3 * CH + M:4 * CH],
                              in_=osb_b[:])

    if legalize:
        _legalize_waits(nc, mybir)
    return nc


def _legalize_waits(nc, mybir):
    """Split multi-wait instructions for the XLA-route walrus codegen.

    The TPB EVENTS struct holds one semaphore wait per instruction and this
    pipeline's codegen refuses >1. Hoist extra waits onto standalone
    EventSemaphore instructions on the same engine queue right before the
    instruction - semantically identical, the queue stalls there.
    """
    n = 0
    for f in nc.m.functions:
        for b in f.blocks:
            out = []
            changed = False
            for inst in b.instructions:
                si = inst.sync_info
                waits = list(si.on_wait) if si is not None and si.on_wait else []
                if len(waits) > 1:
                    changed = True
                    for w in waits[:-1]:
                        n += 1
                        out.append(mybir.InstEventSemaphore(
                            name=f"waitfix{n}_{inst.name}",
                            engine=inst.engine,
                            sync_info=mybir.SyncInfo(on_wait=[w], on_update=[]),
                        ))
                    inst.sync_info = mybir.SyncInfo(
                        on_wait=waits[-1:],
                        on_update=list(si.on_update or []),
                    )
                out.append(inst)
            if changed:
                b.instructions = out
    return n


def kernel(x, wq, bq, wk, bk, wv, bv):
    global LAST_RESULTS
    import os
    os.environ.setdefault("JAX_PLATFORMS", "")
    import ml_dtypes
    from concourse.bass_utils import run_bass_kernel_spmd

    FP8 = ml_dtypes.float8_e4m3
    x = np.asarray(x, dtype=np.float32)
    s = np.float32(1.0 / np.sqrt(HD))
    # per partition p (= row of the D-contraction tile), d-major columns
    w1 = np.concatenate([np.asarray(wq, np.float32) * s,
                         np.asarray(wk, np.float32)], 0).T  # (D, 128)
    w1d = np.ascontiguousarray(
        w1.reshape(DT, P, P).transpose(1, 0, 2)
        .reshape(P, DT * P)).astype(np.float16)
    wv_t = np.asarray(wv, np.float32).T                      # (D, 64)
    wvd = np.ascontiguousarray(
        wv_t.reshape(DT, P, HD).transpose(1, 0, 2)
        .reshape(P, DT * HD)).astype(np.float16)
    b1 = np.concatenate([np.asarray(bq, np.float32) * s,
                         np.asarray(bk, np.float32)])
    bv_f = np.asarray(bv, np.float32)
    bb = np.ascontiguousarray(
        np.stack([b1, np.concatenate([bv_f, bv_f])], axis=1))  # (P, 2)
    # xp[b]: partition-major, chunk-major, d-major: row p holds, for each
    # chunk ci and d-tile d, the 512 values x[b, ci*CH:(ci+1)*CH, d*P+p].
    xp32 = np.ascontiguousarray(
        x.reshape(B, NCH, CH, DT, P).transpose(0, 4, 1, 3, 2)
        .reshape(B, P, NCH, DT, CH))          # (B, P, ci, d, CH) f32
    xp8 = xp32.astype(FP8)
    xp16 = xp32.astype(np.float16)

    def v8(a):
        return np.ascontiguousarray(a).view(FP8).reshape(P, -1)

    wxa_b, wxb_b = [], []
    for b in range(B):
        x8a = v8(xp8[b, :, :, :DH])           # chunks-major, d 0-3
        x8b = v8(xp8[b, :, :, DH:])
        xfa = v8(xp16[b, :, :, :DH])
        xfb = v8(xp16[b, :, :, DH:])
        wxa_b.append(np.ascontiguousarray(np.concatenate(
            [v8(w1d[:, :DH * P]), v8(bb), x8a, xfa], axis=1)))
        wxb_b.append(np.ascontiguousarray(np.concatenate(
            [v8(w1d[:, DH * P:]), v8(wvd), x8b, xfb], axis=1)))
    assert wxa_b[0].shape == (P, A_TOT), wxa_b[0].shape
    assert wxb_b[0].shape == (P, B_TOT), wxb_b[0].shape

    nc = _build_module()
    in_maps = [
        {"wxa": wxa_b[b], "wxb": wxb_b[b]}
        for b in range(B)
    ]
    res = None
    for attempt in range(3):
        try:
            res = run_bass_kernel_spmd(nc, in_maps, core_ids=list(range(B)))
            break
        except Exception:
            # transient device wedges (NRT_EXEC_UNIT_UNRECOVERABLE) happen;
            # rebuild the module and retry on a clean execution
            if attempt == 2:
                raise
            nc = _build_module()
    LAST_RESULTS = res

    out = np.empty((B, T, HD), dtype=np.float32)
    for b in range(B):
        # (65, T) f16: rows 0..63 = O^T, row 64 = denom; divide in f32
        oT = np.asarray(res.results[b]["outT"], dtype=np.float32)
        out[b] = (oT[:HD] / oT[HD:HD + 1]).T
    return out
